# revision 36
# baseline (speedup 1.0000x reference)
"""Trainium2 Bass kernel for nn_Attention_C (XCA-style channel attention block).

Pipeline per image: 1x1 conv (GEMM) -> depthwise 3x3 conv -> per-head
l2norm + channel-attention (48x48 Gram over 4096 pixels) -> softmax ->
attn @ v -> 1x1 out-projection.

Sharding: data-parallel over batch. 16 images / 8 cores = 2 images per core.

v2 layout/schedule notes:
  - channel-major [chan(partition), pixel(free)] everywhere except the
    pixel-major q/k copies used by the Gram matmuls
  - the depthwise conv runs on per-half padded buffers (34 rows x 66 cols,
    1-pixel halo) so all 9 taps are plain free-dim offset views; work is
    split across TensorE (diagonal-weight matmuls) and DVE/GpSimd
    (tensor_scalar + scalar_tensor_tensor chains), tuned for balance
  - GEMM evictions are 1024 columns per ACTIVATE (2 PSUM banks) to
    amortize the ~220-cycle ACT access overhead
  - q/k are scaled by rsqrt(norm) (and temperature) per channel *before*
    the pixel-major transpose, so the Gram output feeds softmax directly
  - v is gathered from conv output straight into the head-padded SBUF
    layout (no DRAM roundtrip)
  - emission order pipelines the two images: img1's qkv GEMM overlaps
    img0's attention tail
"""

import os
import sys
import types

import numpy as np

_REPO = "/opt/trn_rl_repo"
if _REPO not in sys.path:
    sys.path.insert(0, _REPO)

# ---------------------------------------------------------------------------
# antenv.axon_hooks shim (the image's antenv lacks it; needed for trace=True)
# ---------------------------------------------------------------------------
if "antenv.axon_hooks" not in sys.modules:
    try:
        from trn_agent_boot.trn_boot import _ntff_profile_via_ctypes

        _hook = _ntff_profile_via_ctypes("/opt/axon/libaxon_pjrt.so")
    except Exception:
        _hook = None
    _m = types.ModuleType("antenv.axon_hooks")
    _m.get_axon_ntff_profile_hook = lambda: _hook
    _m.set_axon_ntff_profile_hook = lambda h: None
    sys.modules["antenv.axon_hooks"] = _m

import ml_dtypes  # noqa: E402
import bass_rust  # noqa: E402
import concourse.bass as bass  # noqa: E402
import concourse.mybir as mybir  # noqa: E402
import concourse.tile as tile  # noqa: E402
from concourse.bass_utils import run_bass_kernel_spmd  # noqa: E402
from concourse.masks import make_identity  # noqa: E402

BF16 = mybir.dt.bfloat16
F8 = mybir.dt.float8e4
F32 = mybir.dt.float32
AF = mybir.ActivationFunctionType
ALU = mybir.AluOpType
AX = mybir.AxisListType

# ---------------------------------------------------------------------------
# Patch TileContext._drain_and_barrier: this walrus build rejects >1 sync
# waits on a CTRL-class (Drain) instruction; split them into standalone waits.
# ---------------------------------------------------------------------------
_MAX_DRAIN_WAITS = 1


def _split_drain_and_barrier(self, tick_clock, wait_clock):
    from concourse.tile import ScopedClock

    nc = self.nc
    drain_inst = nc.sync.drain()
    wait_clock.add_sem_waits(
        drain_inst.ins, ScopedClock({None: tick_clock.global_clock})
    )
    waits = list(drain_inst.ins.sync_info.on_wait)
    if len(waits) > _MAX_DRAIN_WAITS:
        assert self.sems is not None
        by_num = {h.num: h for h in self.sems.allocated().values()}
        keep, spill = [], []
        for w in waits:
            if w.sync_type == "semaphore" and w.id in by_num:
                spill.append(w)
            else:
                keep.append(w)
        while spill and len(keep) < _MAX_DRAIN_WAITS:
            keep.append(spill.pop())
        drain_inst.ins.sync_info = bass_rust.SyncInfo(on_wait=keep, on_update=[])
        for w in spill:
            nc.sync.wait_ge(by_num[w.id], int(w.wait_value))

    nc.all_engine_barrier()
    assert self.sems is not None
    popped = nc._tile_sem_poison_stack.pop()
    assert popped is self._sem_poison
    nc.clear_and_free_semaphores(list(self.sems.allocated().values()))
    nc.all_engine_barrier()


tile.TileContext._drain_and_barrier = _split_drain_and_barrier


def _split_sync_waits(nc, max_waits=1, max_updates=1):
    """walrus rejects instructions with too many sync wait/update commands;
    spill excess waits onto preceding same-engine NoOps (and excess updates
    onto following ones)."""
    for f in nc.m.functions:
        for bb in f.blocks:
            il = list(bb.instructions)
            out = []
            changed = False
            for inst in il:
                si = inst.sync_info
                if si is None:
                    out.append(inst)
                    continue
                waits = list(si.on_wait)
                ups = list(si.on_update)
                pre, post = [], []
                if len(waits) > max_waits:
                    keep = waits[:max_waits]
                    for i in range(max_waits, len(waits), max_waits):
                        n = mybir.InstNoOp(
                            name=f"I-sw{nc.next_id()}", ins=[], outs=[])
                        n.engine = inst.engine
                        n.sync_info = bass_rust.SyncInfo(
                            on_wait=waits[i : i + max_waits], on_update=[])
                        pre.append(n)
                    changed = True
                else:
                    keep = waits
                if len(ups) > max_updates:
                    kup = ups[:max_updates]
                    for i in range(max_updates, len(ups), max_updates):
                        n = mybir.InstNoOp(
                            name=f"I-su{nc.next_id()}", ins=[], outs=[])
                        n.engine = inst.engine
                        n.sync_info = bass_rust.SyncInfo(
                            on_wait=[], on_update=ups[i : i + max_updates])
                        post.append(n)
                    changed = True
                else:
                    kup = ups
                if pre or post:
                    inst.sync_info = bass_rust.SyncInfo(
                        on_wait=keep, on_update=kup)
                out.extend(pre)
                out.append(inst)
                out.extend(post)
            if changed:
                bb.instructions = out

# ---------------------------------------------------------------------------
# Problem constants (hardcoded; spec: x [16, 384, 64, 64] f32, 8 heads)
# ---------------------------------------------------------------------------
NCORES = 8
BTOT, C, H, W = 16, 384, 64, 64
HEADS = 8
CP = C // HEADS  # 48
C3 = 3 * C  # 1152
NPIX = H * W  # 4096
B = BTOT // NCORES  # images per core

P = 128
RS = W + 2  # padded row stride 66
HH = H // 2  # rows per half (32)
HBLEN = RS * (HH + 2)  # half-buffer length 2244
HINT0 = RS + 1  # first interior position in a half buffer (67)
HLEN = RS * HH - 2  # interior span covering all real pixels of a half (2110)
NST = C3 // P  # 9 channel subtiles of qkv
KT = NPIX // P  # 32 gram contraction tiles
NPG = HEADS // 2  # 4 head-pair groups (v / attn-out layout)
NPR = HEADS // 2  # 4 gram head-pairs

# conv taps: offset in padded layout, tap index (kh, kw) row-major
TAPS = [(RS * (kh - 1) + (kw - 1), 3 * kh + kw) for kh in range(3)
        for kw in range(3)]

# conv work assignment: (img, st, half) -> engine.
# Rough per-half costs: PE 7.9us (+3.4us ACT evict), DVE ~21us, GP ~42us.
# Per group of 6 halves aim: PE 3, DVE 2, GP 1.


# fp8 DoubleRow tap pairs for the q/k conv. The second stream of each
# pair reads a duplicate copy of the slot at +HBLEN, so the pair stride
# is HBLEN+2 (large strides follow the standard DoubleRow layout; small
# overlapping strides crash the device). Pair deltas are all +2 and the
# bases are even, keeping both streams 2-byte aligned.
# Pairs: (-67,-65) (-1,+1) (+65,+67); taps -66, 0, +66 run as plain
# fp8 matmuls.
TAP_PAIRS = [(-RS - 1, 2), (-1, 2), (RS - 1, 2)]
TAP_SINGLES = [-RS, 0, RS]  # dm slots (3,0), (3,1), (4,0)


def _conv_assignment():
    a = {}
    for img in range(B):
        # q/k halves (fp8) all run on PE; v (bf16) splits PE/DVE
        for st in range(6):
            a[(img, st, 0)] = "pe"
            a[(img, st, 1)] = "pe"
        a[(img, 6, 0)] = "pe"
        a[(img, 6, 1)] = "dve"
        a[(img, 7, 0)] = "dve"
        a[(img, 7, 1)] = "pe"
        a[(img, 8, 0)] = "dve"
        a[(img, 8, 1)] = "dve"
    return a


CONV_ASSIGN = _conv_assignment()

# v gather: head h occupies partitions 64*(h%2) .. +48 of padded group h//2.
# Source: conv stage for subtile st covers channels [128*(st-6), 128*(st-6)+128)
# of v (v chans 0..383). Pieces per st: (src_lo, src_hi, dst_part, grp)


def _v_pieces(st):
    base = P * (st - 6)
    pieces = []
    for h in range(HEADS):
        lo, hi = CP * h, CP * h + CP
        a, b = max(lo, base), min(hi, base + P)
        if a >= b:
            continue
        pieces.append((a - base, b - base, 64 * (h % 2) + (a - lo), h // 2))
    return pieces


V_PIECES = {st: _v_pieces(st) for st in (6, 7, 8)}


def _build_nc():
    nc = bass.Bass("TRN2", target_bir_lowering=False, debug=False,
                   num_devices=NCORES)

    # ---- DRAM tensors (host pre-arranged to SBUF-shaped layouts) ----
    x_d = nc.dram_tensor("x", [B, C // P, P, NPIX], BF16, kind="ExternalInput")
    x8_d = nc.dram_tensor("x8", [B, C // P, P, NPIX], F8, kind="ExternalInput")
    wq8_d = nc.dram_tensor("wq8", [P, C // P, 2 * C], F8, kind="ExternalInput")
    wqv_d = nc.dram_tensor("wqv", [P, C // P, C], BF16, kind="ExternalInput")
    diag8_d = nc.dram_tensor("diag8", [6, 5, 2, P, P], F8,
                             kind="ExternalInput")
    wo_d = nc.dram_tensor("woT", [P, NPG, C], BF16, kind="ExternalInput")
    bq_d = nc.dram_tensor("bq", [P, NST], F32, kind="ExternalInput")
    bdw_d = nc.dram_tensor("bdw", [P, NST], F32, kind="ExternalInput")
    bo_d = nc.dram_tensor("bo", [P, C // P], F32, kind="ExternalInput")
    dww_d = nc.dram_tensor("dww", [P, NST, 9], F32, kind="ExternalInput")
    tmpc_d = nc.dram_tensor("tmpc", [P, C // P], F32, kind="ExternalInput")
    diag_d = nc.dram_tensor("diag", [3, 9, P, P], BF16, kind="ExternalInput")
    y_d = nc.dram_tensor("y", [B, C // P, P, NPIX], F32, kind="ExternalOutput")

    with tile.TileContext(nc) as tc:
        with (
            tc.tile_pool(name="consts", bufs=1) as consts,
            tc.tile_pool(name="slots", bufs=1) as slot_pool,
            tc.tile_pool(name="stage", bufs=3) as stage_pool,
            tc.tile_pool(name="qkc", bufs=2) as qkc_pool,
            tc.tile_pool(name="xv", bufs=2) as xv_pool,
            tc.tile_pool(name="diagw", bufs=2) as diag_pool,
            tc.tile_pool(name="perimg", bufs=1) as perimg,
            tc.tile_pool(name="smalls", bufs=1) as smalls,
            tc.tile_pool(name="ao", bufs=1) as ao_pool,
            tc.tile_pool(name="yt", bufs=2) as yt_pool,
            tc.tile_pool(name="psA", bufs=2, space="PSUM") as psA,
            tc.tile_pool(name="psB", bufs=2, space="PSUM") as psB,
            tc.tile_pool(name="psG", bufs=1, space="PSUM") as psG,
            tc.tile_pool(name="psT", bufs=1, space="PSUM") as psT,
        ):
            # ---- load constants ----
            wq8 = consts.tile([P, C // P, 2 * C], F8)
            nc.sync.dma_start(out=wq8, in_=wq8_d[:])
            wqv = consts.tile([P, C // P, C], BF16)
            nc.sync.dma_start(out=wqv, in_=wqv_d[:])
            wo = consts.tile([P, NPG, C], BF16)
            nc.sync.dma_start(out=wo, in_=wo_d[:])
            bq = consts.tile([P, NST], F32)
            nc.sync.dma_start(out=bq, in_=bq_d[:])
            bdw = consts.tile([P, NST], F32)
            nc.sync.dma_start(out=bdw, in_=bdw_d[:])
            bo = consts.tile([P, C // P], F32)
            nc.sync.dma_start(out=bo, in_=bo_d[:])
            dww = consts.tile([P, NST, 9], F32)
            nc.sync.dma_start(out=dww, in_=dww_d[:])
            tmpc = consts.tile([P, C // P], F32)
            nc.sync.dma_start(out=tmpc, in_=tmpc_d[:])
            ident = consts.tile([P, P], F32)
            make_identity(nc, ident)

            # ---- persistent buffers ----
            # dedicated-role half slots: A = top-half (top halo zero), B =
            # bottom-half (bottom halo zero); 2-col gaps zero in both.
            # q/k slots are fp8 (ring of 3 per role), v slots bf16 (ring 2).
            slabA = [slot_pool.tile([P, 2, HBLEN], F8, tag=f"slA{i}",
                                    name=f"slA{i}") for i in range(3)]
            slabB = [slot_pool.tile([P, 2, HBLEN], F8, tag=f"slB{i}",
                                    name=f"slB{i}") for i in range(3)]
            slabAv = [slot_pool.tile([P, HBLEN], BF16, tag=f"slAv{i}",
                                     name=f"slAv{i}") for i in range(2)]
            slabBv = [slot_pool.tile([P, HBLEN], BF16, tag=f"slBv{i}",
                                     name=f"slBv{i}") for i in range(2)]
            slabA0 = [sl[:, 0] for sl in slabA]
            slabB0 = [sl[:, 0] for sl in slabB]
            for sl in slabA0 + slabB0 + slabAv + slabBv:
                # left/right pad columns of every row (cols 65, 66 adjacent)
                gaps = bass.AP(
                    tensor=sl.tensor,
                    offset=sl.offset + (RS - 1),
                    ap=[list(sl.ap[0]), [RS, HH + 1], [1, 2]],
                )
                nc.gpsimd.memset(gaps, 0.0)
                nc.gpsimd.memset(sl[:, 0:1], 0.0)
            for sl in slabA0 + slabAv:
                nc.gpsimd.memset(sl[:, 1 : RS], 0.0)  # top halo row
                # bottom-right corner cell, read by the +RS+1 tap
                nc.gpsimd.memset(sl[:, HBLEN - 1 :], 0.0)
            for sl in slabB0 + slabBv:
                nc.gpsimd.memset(sl[:, RS * (HH + 1) + 1 :], 0.0)  # bottom halo

            xfull8 = perimg.tile([P, C // P, NPIX], F8, tag="xfull8")
            vpad = perimg.tile([P, NPG, NPIX], BF16, tag="vpad")
            lhsav = perimg.tile([P, NPG, P], BF16, tag="lhsav")
            nc.gpsimd.memset(vpad, 0.0)
            nc.gpsimd.memset(lhsav, 0.0)

            # shared across images: img1's transposes WAR-wait on img0's gram
            qTt = perimg.tile([P, KT, C], BF16, tag="qTt", name="qTt")
            kTt = perimg.tile([P, KT, C], BF16, tag="kTt", name="kTt")
            qT = {0: qTt, 1: qTt}
            kT = {0: kTt, 1: kTt}

            n2 = {}
            for img in range(B):
                # sum of squares per channel, q sts 0-2 / k sts 3-5
                n2[img] = perimg.tile([P, 6], F32, tag=f"n2_{img}", name=f"n2_{img}")

            slot_of = {}  # (img, st, half) -> slab tile
            slab_ri = {}  # (img, st) -> ring slot key
            ring_idx = [0]
            ring_v = [0]

            # ----------------------------------------------------------------
            # emission helpers
            # ----------------------------------------------------------------
            def emit_xload(img):
                for c4 in range(4):
                    nc.scalar.dma_start(
                        out=xfull8[:, :, 1024 * c4 : 1024 * c4 + 1024],
                        in_=x8_d[img, :, :, 1024 * c4 : 1024 * c4 + 1024]
                        .rearrange("k p n -> p k n"),
                    )

            def emit_gemm(img, st):
                """qkv GEMM for subtile st into padded half slots.

                q/k subtiles (st<6) run fp8 with DoubleRow (16x-scaled
                weights, scale=1/16 on eviction); v subtiles run bf16.
                """
                qk = st < 6
                if qk:
                    ri = ring_idx[0] % 3
                    ring_idx[0] += 1
                    tA, tB = slabA[ri], slabB[ri]
                    hA, hB = slabA0[ri], slabB0[ri]
                    rkey = ("qk", ri)
                else:
                    ri = ring_v[0] % 2
                    ring_v[0] += 1
                    tA = tB = None
                    hA, hB = slabAv[ri], slabBv[ri]
                    rkey = ("v", ri)
                # conv of the slab's previous tenant must be fully emitted
                flush(lambda e: e["slab_ri"] == rkey)
                slab_ri[(img, st)] = rkey
                slot_of[(img, st, 0)] = hA
                slot_of[(img, st, 1)] = hB
                for nt2 in range(4):
                    ps = psA.tile([P, 1024], F32, tag="gemm")
                    if not qk:
                        xv = xv_pool.tile([P, C // P, 1024], BF16, tag="xv",
                                          name="xv")
                        nc.sync.dma_start(
                            out=xv,
                            in_=x_d[img, :, :,
                                    1024 * nt2 : 1024 * nt2 + 1024]
                            .rearrange("k p n -> p k n"),
                        )
                    for hf in range(2):
                        o = 1024 * nt2 + 512 * hf
                        pso = ps[:, 512 * hf : 512 * hf + 512]
                        if qk:
                            nc.tensor.matmul(
                                pso,
                                wq8[:, 0:2, P * st : P * st + P],
                                xfull8[:, 0:2, o : o + 512],
                                perf_mode=mybir.MatmulPerfMode.DoubleRow,
                                start=True,
                                stop=False,
                            )
                            nc.tensor.matmul(
                                pso,
                                wq8[:, 2, P * st : P * st + P],
                                xfull8[:, 2, o : o + 512],
                                start=False,
                                stop=True,
                            )
                        else:
                            for k in range(C // P):
                                nc.tensor.matmul(
                                    pso,
                                    wqv[:, k, P * (st - 6) : P * (st - 6) + P],
                                    xv[:, k,
                                       512 * hf : 512 * hf + 512],
                                    start=(k == 0),
                                    stop=(k == C // P - 1),
                                )
                    sc = (1.0 / 16.0) if qk else 1.0
                    # rows 16*nt2 .. +16 of the image; halves split at row 32
                    half = nt2 // 2
                    dst = hA if half == 0 else hB
                    r0 = (16 * nt2) % 32  # row within the half
                    dest = bass.AP(
                        tensor=dst.tensor,
                        offset=dst.offset + RS * (r0 + 1) + 1,
                        ap=[list(dst.ap[0]), [RS, 16], [1, W]],
                    )
                    nc.scalar.activation(
                        out=dest,
                        in_=ps.rearrange("p (r w) -> p r w", w=W),
                        func=AF.Identity,
                        bias=bq[:, st : st + 1],
                        scale=sc,
                    )
                    if nt2 == 1:
                        # image row 31 -> half B halo row 0
                        nc.scalar.activation(
                            out=bass.AP(
                                tensor=hB.tensor,
                                offset=hB.offset + 1,
                                ap=[list(hB.ap[0]), [1, W]],
                            ),
                            in_=ps[:, 15 * W : 16 * W],
                            func=AF.Identity,
                            bias=bq[:, st : st + 1],
                            scale=sc,
                        )
                    elif nt2 == 2:
                        # image row 32 -> half A halo row 33
                        nc.scalar.activation(
                            out=bass.AP(
                                tensor=hA.tensor,
                                offset=hA.offset + RS * (HH + 1) + 1,
                                ap=[list(hA.ap[0]), [1, W]],
                            ),
                            in_=ps[:, 0:W],
                            func=AF.Identity,
                            bias=bq[:, st : st + 1],
                            scale=sc,
                        )
                if qk:
                    # duplicate each half for the DoubleRow pair stride
                    nc.gpsimd.dma_start(out=tA[:, 1], in_=tA[:, 0])
                    nc.gpsimd.dma_start(out=tB[:, 1], in_=tB[:, 0])

            # ---- pumped PE-conv queue: chunks interleave with GEMM blocks
            # on the PE queue so a stall on one PSUM pool hides under work
            # on the other ----
            pe_queue = []  # dicts: key=(img, st), slab_ri, fn
            outstanding = {}  # key -> un-emitted PE conv steps
            post_fn = {}  # key -> post closure once conv fully emitted

            def _try_post(key):
                if outstanding.get(key, 0) == 0 and key in post_fn:
                    post_fn.pop(key)()

            def _pump_one():
                ent = pe_queue.pop(0)
                ent["fn"]()
                key = ent["key"]
                outstanding[key] -= 1
                _try_post(key)

            def pump(n=1):
                for _ in range(min(n, len(pe_queue))):
                    _pump_one()

            def flush(pred):
                last = -1
                for idx, ent in enumerate(pe_queue):
                    if pred(ent):
                        last = idx
                for _ in range(last + 1):
                    _pump_one()

            def emit_conv_dve(img, st, hf, stage):
                slot = slot_of[(img, st, hf)]
                a = HINT0
                for toff, t in TAPS:
                    src = slot[:, a + toff : a + toff + HLEN]
                    wsc = dww[:, st, t : t + 1]
                    if t == 0:
                        nc.vector.tensor_scalar(
                            out=stage, in0=src,
                            scalar1=wsc, scalar2=bdw[:, st : st + 1],
                            op0=ALU.mult, op1=ALU.add,
                        )
                    else:
                        nc.vector.scalar_tensor_tensor(
                            out=stage, in0=src,
                            scalar=wsc, in1=stage,
                            op0=ALU.mult, op1=ALU.add,
                        )

            def conv_pe_steps(img, st, hf, stage):
                slot = slot_of[(img, st, hf)]
                a = HINT0
                qk = st < 6
                dmref = {}

                def mkstep(off, n, first):
                    def step():
                        if first:
                            if qk:
                                dm = diag_pool.tile([P, 5, 2, P], F8,
                                                    tag="dg8", name="dm8")
                                nc.sync.dma_start(
                                    out=dm,
                                    in_=diag8_d[st]
                                    .rearrange("j i p q -> p j i q"))
                            else:
                                dm = diag_pool.tile([P, 9, P], BF16,
                                                    tag="dg", name="dm")
                                nc.sync.dma_start(
                                    out=dm,
                                    in_=diag_d[st - 6]
                                    .rearrange("t p q -> p t q"))
                            dmref["dm"] = dm
                        dm = dmref["dm"]
                        ps = psB.tile([P, 512], F32, tag="conv", name="cps")
                        if qk:
                            # 3 DoubleRow pairs (second stream reads the
                            # +HBLEN duplicate copy) + 3 plain fp8 taps
                            for j, (t0, dlt) in enumerate(TAP_PAIRS):
                                rhs = bass.AP(
                                    tensor=slot.tensor,
                                    offset=slot.offset + a + off + t0,
                                    ap=[list(slot.ap[0]),
                                        [HBLEN + dlt, 2], [1, n]],
                                )
                                nc.tensor.matmul(
                                    ps[:, :n],
                                    dm[:, j, :, :],
                                    rhs,
                                    perf_mode=mybir.MatmulPerfMode.DoubleRow,
                                    start=(j == 0),
                                    stop=False,
                                )
                            for j, toff in enumerate(TAP_SINGLES):
                                o = a + off + toff
                                nc.tensor.matmul(
                                    ps[:, :n],
                                    dm[:, 3 + j // 2, j % 2, :],
                                    slot[:, o : o + n],
                                    start=False,
                                    stop=(j == 2),
                                )
                        else:
                            for toff, t in TAPS:
                                nc.tensor.matmul(
                                    ps[:, :n],
                                    dm[:, t, :],
                                    slot[:, a + off + toff :
                                         a + off + toff + n],
                                    start=(t == 0),
                                    stop=(t == 8),
                                )
                        if qk:
                            # DVE is idle during q/k phases; evict there to
                            # keep ACT off the PE's critical path
                            nc.vector.tensor_scalar(
                                out=stage[:, off : off + n],
                                in0=ps[:, :n],
                                scalar1=1.0 / 16.0,
                                scalar2=bdw[:, st : st + 1],
                                op0=ALU.mult, op1=ALU.add,
                            )
                        else:
                            nc.scalar.activation(
                                out=stage[:, off : off + n],
                                in_=ps[:, :n],
                                func=AF.Identity,
                                bias=bdw[:, st : st + 1],
                            )
                    return step

                steps = []
                off = 0
                while off < HLEN:
                    n = min(512, HLEN - off)
                    steps.append(mkstep(off, n, off == 0))
                    off += n
                return steps

            def schedule_conv(img, st, post):
                """emit DVE halves now; enqueue PE halves; register post."""
                flush_scales()  # previous subtile's deferred norm chain
                key = (img, st)
                stages = []
                nq = 0
                for hf in range(2):
                    stage = stage_pool.tile([P, HLEN], BF16, tag="stage",
                                            name="stage")
                    stages.append(stage)
                    if CONV_ASSIGN[(img, st, hf)] == "pe":
                        for fn in conv_pe_steps(img, st, hf, stage):
                            fn()
                    else:
                        emit_conv_dve(img, st, hf, stage)
                outstanding[key] = nq
                post_fn[key] = lambda: post(stages)
                _try_post(key)

            def stage_rows_ap(stage):
                # interior rows of a conv stage: row r (0..31) at 66*r, 64 wide
                return bass.AP(
                    tensor=stage.tensor,
                    offset=stage.offset,
                    ap=[list(stage.ap[0]), [RS, HH], [1, W]],
                )

            sqscratch = perimg.tile([P, 2048], BF16, tag="sqscratch")

            pending_scale = []  # deferred DVE norm-chain closures

            def flush_scales():
                while pending_scale:
                    pending_scale.pop(0)()

            def emit_qk_post2(img, st, stages, qkdst):
                grp = st // 3
                s = st - 3 * grp
                ncol = 3 * grp + s  # n2 column (q: 0-2, k: 3-5)
                cdst = qkc_pool.tile([P, NPIX], BF16, tag="qkc")
                for hf, stage in enumerate(stages):
                    nc.sync.dma_start(
                        out=cdst[:, 2048 * hf : 2048 * hf + 2048]
                        .rearrange("p (r w) -> p r w", w=W),
                        in_=stage_rows_ap(stage),
                    )
                # sum of squares per channel; squares land in scratch.
                # Split in 2 chunks so conv/GEMM evictions interleave on ACT.
                n2p = smalls.tile([P, 2], F32, tag="n2p")
                for hf in range(2):
                    nc.scalar.activation(
                        out=sqscratch,
                        in_=cdst[:, 2048 * hf : 2048 * hf + 2048],
                        func=AF.Square,
                        accum_out=n2p[:, hf : hf + 1],
                    )

                def scale_chain():
                    nc.vector.tensor_tensor(
                        out=n2[img][:, ncol : ncol + 1], in0=n2p[:, 0:1],
                        in1=n2p[:, 1:2], op=ALU.add,
                    )
                    # rn = 1/sqrt(n2); scale: q: rn * temp, k: rn
                    rn = smalls.tile([P, 1], F32, tag="rn", name="rn")
                    nc.scalar.activation(
                        out=rn, in_=n2[img][:, ncol : ncol + 1], func=AF.Sqrt)
                    nc.vector.reciprocal(out=rn, in_=rn)
                    if grp == 0:
                        nc.vector.tensor_scalar(
                            out=cdst, in0=cdst,
                            scalar1=rn, scalar2=tmpc[:, s : s + 1],
                            op0=ALU.mult, op1=ALU.mult,
                        )
                    else:
                        nc.vector.tensor_scalar_mul(cdst, cdst, rn)
                    # pixel-major transpose (split across SP/ACT queues)
                    eng = nc.sync if (st % 2 == 0) else nc.scalar
                    eng.dma_start_transpose(
                        qkdst[:, :, P * s : P * s + P], cdst[:])

                pending_scale.append(scale_chain)

            def emit_v_post(img, st, stages):
                for hf, stage in enumerate(stages):
                    for (slo, shi, dlo, g) in V_PIECES[st]:
                        sub = stage[slo:shi]  # partition slice
                        src = bass.AP(
                            tensor=sub.tensor,
                            offset=sub.offset,
                            ap=[list(sub.ap[0]), [RS, HH], [1, W]],
                        )
                        nc.gpsimd.dma_start(
                            out=vpad[dlo : dlo + (shi - slo), g,
                                     2048 * hf : 2048 * hf + 2048]
                            .rearrange("p (r w) -> p r w", w=W),
                            in_=src,
                        )

            GRAM_HEADS_BY_J = [[0, 1], [2, 3, 4], [5, 6, 7]]

            def emit_group(img, grp, mid=None, gram=None):
                sts = [3 * grp + i for i in range(3)]
                if gram is not None:
                    S = smalls.tile([CP, HEADS, CP], F32, tag="S", name="S")
                    Sref[img] = S

                def mkpost(st):
                    if grp < 2:
                        qkdst = qT[img] if grp == 0 else kT[img]
                        return lambda stages: emit_qk_post2(
                            img, st, stages, qkdst)
                    return lambda stages: emit_v_post(img, st, stages)

                for j, st in enumerate(sts):
                    emit_gemm(img, st)
                    schedule_conv(img, st, mkpost(st))
                    if j == 1 and mid is not None:
                        mid()
                    if gram is not None and j >= 1:
                        # heads of the previous k-subtile (transpose landed)
                        flush_scales()
                        for h in GRAM_HEADS_BY_J[j - 1]:
                            emit_gram_head(img, Sref[img], h)
                if gram is not None:
                    flush_scales()
                    for h in GRAM_HEADS_BY_J[2]:
                        emit_gram_head(img, Sref[img], h)

            def emit_gram_head(img, S, h):
                ps = psG.tile([CP, CP], F32, tag="gram", name="gps")
                for kt in range(KT):
                    nc.tensor.matmul(
                        ps,
                        qT[img][:, kt, CP * h : CP * h + CP],
                        kT[img][:, kt, CP * h : CP * h + CP],
                        start=(kt == 0),
                        stop=(kt == KT - 1),
                    )
                nc.vector.tensor_copy(out=S[:, h, :], in_=ps)

            def emit_attn_tail(img, S):
                flush(lambda e: e["key"][0] == img)
                flush_scales()
                # ---- batched softmax over last dim ----
                mx = smalls.tile([CP, HEADS], F32, tag="mx")
                nc.vector.tensor_reduce(out=mx, in_=S, axis=AX.X, op=ALU.max)
                nc.vector.tensor_tensor(
                    out=S, in0=S, in1=mx[:, :, None].to_broadcast(S.shape),
                    op=ALU.subtract,
                )
                nc.scalar.activation(out=S, in_=S, func=AF.Exp)
                sm = smalls.tile([CP, HEADS], F32, tag="sm")
                nc.vector.tensor_reduce(out=sm, in_=S, axis=AX.X, op=ALU.add)
                nc.vector.reciprocal(out=sm, in_=sm)
                nc.vector.tensor_tensor(
                    out=S, in0=S, in1=sm[:, :, None].to_broadcast(S.shape),
                    op=ALU.mult,
                )

                # ---- transpose attn per head, assemble AV lhsT blocks ----
                for h in range(HEADS):
                    pst = psT.tile([CP, CP], F32, tag="tr")
                    nc.tensor.transpose(pst, S[:, h, :], ident[0:CP, 0:CP])
                    o = 64 * (h % 2)
                    nc.vector.tensor_copy(
                        out=lhsav[o : o + CP, h // 2, o : o + CP], in_=pst,
                    )

                # ---- attn @ v, then out-projection, per pixel tile ----
                for nt in range(NPIX // 512):
                    ao = ao_pool.tile([P, NPG, 512], BF16, tag="ao")
                    for g in range(NPG):
                        ps = psB.tile([P, 512], F32, tag="conv")
                        nc.tensor.matmul(
                            ps,
                            lhsav[:, g, :],
                            vpad[:, g, 512 * nt : 512 * nt + 512],
                            start=True,
                            stop=True,
                        )
                        nc.vector.tensor_copy(out=ao[:, g, :], in_=ps)
                    for mo in range(C // P):
                        ps = psB.tile([P, 512], F32, tag="conv")
                        for k in range(NPG):
                            nc.tensor.matmul(
                                ps,
                                wo[:, k, P * mo : P * mo + P],
                                ao[:, k, :],
                                start=(k == 0),
                                stop=(k == NPG - 1),
                            )
                        yt = yt_pool.tile([P, 512], F32, tag="yt")
                        nc.vector.tensor_scalar(
                            out=yt, in0=ps, scalar1=bo[:, mo : mo + 1],
                            scalar2=None, op0=ALU.add,
                        )
                        nc.sync.dma_start(
                            out=y_d[img, mo, :, 512 * nt : 512 * nt + 512],
                            in_=yt,
                        )
                    pump(1)

            # ----------------------------------------------------------------
            # schedule
            # ----------------------------------------------------------------
            Sref = {}
            emit_xload(0)
            emit_group(0, 0)   # q0 (fp8, PE-heavy)
            emit_group(0, 2)   # v0 (bf16, DVE-heavy) overlaps k0's PE work
            emit_group(0, 1, gram=0)  # k0 with gram heads interleaved
            emit_xload(1)
            emit_group(1, 0, mid=lambda: emit_attn_tail(0, Sref[0]))
            emit_group(1, 2)   # v1 overlaps k1
            emit_group(1, 1, gram=1)
            emit_attn_tail(1, Sref[1])
            pump(10**9)

    _split_sync_waits(nc)
    return nc


_CACHE = {}


def kernel(x, W_qkv, b_qkv, W_dw, b_dw, W_out, b_out, temperature):
    x = np.asarray(x, np.float32)
    W_qkv = np.asarray(W_qkv, np.float32)
    b_qkv = np.asarray(b_qkv, np.float32)
    W_dw = np.asarray(W_dw, np.float32)
    b_dw = np.asarray(b_dw, np.float32)
    W_out = np.asarray(W_out, np.float32)
    b_out = np.asarray(b_out, np.float32)
    temperature = np.asarray(temperature, np.float32)

    if "nc" not in _CACHE:
        _CACHE["nc"] = _build_nc()
    nc = _CACHE["nc"]

    # ---- host-side prep into SBUF-shaped layouts ----
    # q/k weights 16x-scaled fp8 (compensated by scale=1/16 at eviction)
    wq8 = np.ascontiguousarray(
        (16.0 * W_qkv[: 2 * C]).T.reshape(C // P, P, 2 * C).transpose(1, 0, 2)
    ).astype(ml_dtypes.float8_e4m3)  # [128, 3, 768]
    wqv = np.ascontiguousarray(
        W_qkv[2 * C :].T.reshape(C // P, P, C).transpose(1, 0, 2)
    ).astype(ml_dtypes.bfloat16)  # [128, 3, 384]
    wpad = np.zeros((4 * P, C), np.float32)  # [512, 384] padded in-chans
    for h in range(HEADS):
        wpad[64 * (h % 2) + 128 * (h // 2) : 64 * (h % 2) + 128 * (h // 2) + CP] = \
            W_out.T[CP * h : CP * h + CP]
    woT = np.ascontiguousarray(
        wpad.reshape(4, P, C).transpose(1, 0, 2)
    ).astype(ml_dtypes.bfloat16)  # [128, 4, 384]
    bq = np.ascontiguousarray(b_qkv.reshape(NST, P).T)  # [128, 9]
    bdw = np.ascontiguousarray(b_dw.reshape(NST, P).T)  # [128, 9]
    bo = np.ascontiguousarray(b_out.reshape(C // P, P).T)  # [128, 3]
    taps = W_dw.reshape(C3, 9)  # [1152, 9] in (kh, kw) order
    dww = np.ascontiguousarray(
        taps.reshape(NST, P, 9).transpose(1, 0, 2)
    )  # [128, 9, 9]
    # per-channel temperature for q subtiles (chan c -> head c // 48)
    tvec = temperature.reshape(HEADS)
    tmpc = np.ascontiguousarray(
        tvec[(np.arange(C) // CP)].reshape(C // P, P).T
    ).astype(np.float32)  # [128, 3]
    ar = np.arange(P)
    # v diag taps, bf16
    diag = np.zeros((3, 9, P, P), np.float32)
    for s in range(3):
        for t in range(9):
            diag[s, t, ar, ar] = taps[2 * C + P * s : 2 * C + P * s + P, t]
    diag = diag.astype(ml_dtypes.bfloat16)
    # q/k diag taps, 16x-scaled fp8. Slots 0-2: DoubleRow pairs
    # (-67,-65) (-1,+1) (+65,+67); slots 3-4: singles -66, 0, +66.
    pair_idx = [(0, 2), (3, 5), (6, 8)]
    single_idx = [(3, 0, 1), (3, 1, 4), (4, 0, 7)]
    diag8 = np.zeros((6, 5, 2, P, P), np.float32)
    for st in range(6):
        for j, (ta, tb) in enumerate(pair_idx):
            diag8[st, j, 0, ar, ar] = 16.0 * taps[P * st : P * st + P, ta]
            diag8[st, j, 1, ar, ar] = 16.0 * taps[P * st : P * st + P, tb]
        for (j, i, t) in single_idx:
            diag8[st, j, i, ar, ar] = 16.0 * taps[P * st : P * st + P, t]
    diag8 = diag8.astype(ml_dtypes.float8_e4m3)

    xr = x.reshape(BTOT, C // P, P, NPIX).astype(ml_dtypes.bfloat16)
    xr8 = x.reshape(BTOT, C // P, P, NPIX).astype(ml_dtypes.float8_e4m3)

    base = {
        "wq8": wq8, "wqv": wqv, "woT": woT, "bq": bq, "bdw": bdw, "bo": bo,
        "dww": dww, "tmpc": tmpc, "diag": diag, "diag8": diag8,
    }
    in_maps = []
    for core in range(NCORES):
        m = dict(base)
        m["x"] = np.ascontiguousarray(xr[B * core : B * core + B])
        m["x8"] = np.ascontiguousarray(xr8[B * core : B * core + B])
        in_maps.append(m)

    res = run_bass_kernel_spmd(nc, in_maps, list(range(NCORES)),
                               trace=bool(os.environ.get("KERNEL_TRACE")))
    if os.environ.get("KERNEL_TRACE"):
        _CACHE["exec_time_ns"] = res.exec_time_ns

    outs = [res.results[c]["y"].reshape(B, C, H, W) for c in range(NCORES)]
    return np.concatenate(outs, axis=0)
